# revision 6
# baseline (speedup 1.0000x reference)
"""Causal self-attention TRN2 kernel.

Full module: x[4,2048,1024] @ W_qkv[1024,3072] -> heads(16, d=64) causal attn
-> @ W_proj[1024,1024].

Sharding: 8 cores = 4 batches x 2 head-groups (8 heads each), tensor-parallel
over heads. Each core computes q/k/v for its 8 heads, causal attention, and a
partial projection (row-sharded W_proj). The two partials per batch are summed
on the host (no on-device collectives).

v2 changes vs v1 (driven by CoreSim cost-model engine occupancy):
  - Weights (wq/wk/wv/wp) are loaded once into SBUF with 24 big DMAs in a
    preload section instead of 256 per-tile DMAs re-issued every block
    (weights-stationary steady state). SP engine busy drops ~50%.
  - x is loaded with ONE gather-DMA per 512-token block ([128, 8c, 512t])
    instead of 8 tile DMAs, issued on the otherwise-idle GpSimd queue.
  - Causal masking no longer zero-fills masked probability columns from DRAM
    (96 DMAs, ~13 MB/pass): the exp activation, the triangular-mask multiply
    and the P@V / scores matmuls are narrowed to the valid causal region.
    Diagonal k-tiles are emitted widest-last (d=3..0) so the PSUM
    accumulation group still starts with a bank-clearing full-width matmul
    (qc=0) and ends with a full-width stop=True matmul.
  - P (probabilities) and V tiles are bf16: halves their SBUF footprint and
    doubles DVE mask-multiply rate. Scores (q@k), y-normalize and proj stay
    f32r. PSUM accumulation is fp32 throughout.
  - Per-pass DMAs (x loads, output stores) are issued from the GpSimd
    queue (SP does nothing in steady state); the head-B half of the y^T
    tile is written in place by a cross-partition-base DVE multiply, so
    no SBUF->SBUF assembly DMA sits on the attention->proj critical path.
  - build_nc(loop=N) wraps the per-pass body in a hardware For_i loop: the
    NEFF stays small and per-pass device time can be measured as
    (wall(loop=N) - wall(loop=1)) / (N-1) without NEFF-load-time pollution.
"""

import numpy as np
import ml_dtypes
from contextlib import ExitStack

import concourse.bass as bass
import concourse.tile as tile
from concourse import mybir, bacc
from concourse.bass_utils import run_bass_kernel_spmd

F32 = mybir.dt.float32
F32R = mybir.dt.float32r
BF16 = mybir.dt.bfloat16
EXP = mybir.ActivationFunctionType.Exp

B, T, C, H, D = 4, 2048, 1024, 16, 64
NCORES = 8
GROUPS = 2            # head groups (tensor-parallel dimension)
HPC = H // GROUPS     # heads per core = 8
FPC = HPC * D         # features per core = 512
SCALE = 1.0 / np.sqrt(D)


def build_nc(loop=0, body_reps=1, loop_hints=False, interleave_qkv=True, pt_bufs=3):
    NC = C // 128     # contraction chunks over C = 8
    NT = T // 128     # token tiles (also k-tiles) = 16
    NQ = T // 512     # query chunks (= pipeline blocks) = 4
    NF = FPC // 128   # feature tiles = head pairs = 4
    NN = C // 512     # proj output column chunks = 2
    npj = 512

    nc = bacc.Bacc("TRN2", debug=False)
    xT_d = nc.dram_tensor("xT", [C, T], F32R, kind="ExternalInput").ap()
    wq_d = nc.dram_tensor("wq", [C, FPC], F32R, kind="ExternalInput").ap()
    wk_d = nc.dram_tensor("wk", [C, FPC], F32R, kind="ExternalInput").ap()
    wv_d = nc.dram_tensor("wv", [C, FPC], F32R, kind="ExternalInput").ap()
    wp_d = nc.dram_tensor("wp", [FPC, C], F32R, kind="ExternalInput").ap()
    mk_d = nc.dram_tensor("trimask", [128, 128], BF16, kind="ExternalInput").ap()
    on_d = nc.dram_tensor("ones64", [1, 64], F32R, kind="ExternalInput").ap()
    ov_d = nc.dram_tensor("onesv", [128, HPC, 1], BF16, kind="ExternalInput").ap()
    out_d = nc.dram_tensor("out", [T, C], F32, kind="ExternalOutput").ap()

    with tile.TileContext(nc) as tc, ExitStack() as ctx:
        p_kt = ctx.enter_context(tc.tile_pool(name="p_kt", bufs=NF))
        p_v65 = ctx.enter_context(tc.tile_pool(name="p_v65", bufs=NT))
        p_const = ctx.enter_context(tc.tile_pool(name="p_const", bufs=1))
        p_w = ctx.enter_context(tc.tile_pool(name="p_w", bufs=NC))
        p_wp = ctx.enter_context(tc.tile_pool(name="p_wp", bufs=NF))
        p_xq = ctx.enter_context(tc.tile_pool(name="p_xq", bufs=2))
        p_qtq = ctx.enter_context(tc.tile_pool(name="p_qtq", bufs=2 * NF))
        p_ytq = ctx.enter_context(tc.tile_pool(name="p_ytq", bufs=2 * NF))
        p_pt = ctx.enter_context(tc.tile_pool(name="p_pt", bufs=pt_bufs))
        p_rec = ctx.enter_context(tc.tile_pool(name="p_rec", bufs=1))
        p_ys = ctx.enter_context(tc.tile_pool(name="p_ys", bufs=2))
        p_osb = ctx.enter_context(tc.tile_pool(name="p_osb", bufs=2))
        # one shared PSUM budget, 8 banks: s 2x2 + y 2 + misc 2
        ps_s = ctx.enter_context(tc.tile_pool(name="ps_s", bufs=2, space="PSUM"))
        ps_y = ctx.enter_context(tc.tile_pool(name="ps_y", bufs=2, space="PSUM"))
        ps_m = ctx.enter_context(tc.tile_pool(name="ps_m", bufs=2, space="PSUM"))

        # ---------------- preload: constants + weights (once) ----------------
        ones_t = p_const.tile([65, 64], F32R, tag="ones")
        nc.sync.dma_start(out=ones_t[64:65, :], in_=on_d[:])
        trimask = p_const.tile([128, 128], BF16, tag="trimask")
        nc.sync.dma_start(out=trimask[:], in_=mk_d[:])

        kt_ = [p_kt.tile([128, T], F32R, tag="kt", name=f"kt{i}") for i in range(NF)]
        v65 = [p_v65.tile([128, HPC, 65], BF16, tag="v65", name=f"v65_{i}")
               for i in range(NT)]
        for t in range(NT):
            nc.sync.dma_start(out=v65[t][:, :, 64:65], in_=ov_d[:])

        wq_sb, wk_sb, wv_sb = [], [], []
        for c in range(NC):
            for w_d, store, wtag in ((wq_d, wq_sb, "wq"), (wk_d, wk_sb, "wk"),
                                     (wv_d, wv_sb, "wv")):
                wt = p_w.tile([128, FPC], F32R, tag=wtag, name=f"{wtag}{c}")
                nc.sync.dma_start(out=wt[:], in_=w_d[c * 128:(c + 1) * 128, :])
                store.append(wt)
        wp_sb = []
        for cf in range(NF):
            wt = p_wp.tile([128, C], F32R, tag="wp", name=f"wp{cf}")
            nc.sync.dma_start(out=wt[:], in_=wp_d[cf * 128:(cf + 1) * 128, :])
            wp_sb.append(wt)

        # xT viewed as [c-chunk, partition, block, token] for per-chunk loads
        xT_4d = xT_d.rearrange("(c p) (n t) -> c p n t", c=NC, n=NQ)

        def dma_xq_tile(n, name):
            t_ = p_xq.tile([128, NC, 512], F32R, tag="xq", name=name)
            for c in range(NC):
                nc.gpsimd.dma_start(out=t_[:, c, :], in_=xT_4d[c, :, n, :])
            return t_

        # block 0 of the first pass loads during preload; each pass then
        # prefetches the next pass's block 0 at its tail (same rotating
        # buffer, so the loop body is address-uniform across iterations)
        xq0_pre = dma_xq_tile(0, "xq0pre")

        # ---------------- per-pass body ----------------
        def emit_body(prefetch_next=False):
            qtq = {}   # (f, qc) -> [128, 512] query quarter
            ytq = {}   # (f, qc) -> [128, 512] attention-out quarter
            xq = {0: xq0_pre}

            def dma_xq(n):
                xq[n] = dma_xq_tile(n, f"xq{n}")

            def q_group(n, f, isq):
                store = wq_sb if isq else wk_sb
                ps = ps_m.tile([128, 512], F32, tag="m1", name=f"qk{n}_{f}")
                for c in range(NC):
                    nc.tensor.matmul(
                        ps[:], store[c][:, f * 128:(f + 1) * 128],
                        xq[n][:, c, :],
                        start=(c == 0), stop=(c == NC - 1))
                if isq:
                    dst = p_qtq.tile([128, 512], F32R, tag="qt",
                                     name=f"qtq{f}_{n}")
                    qtq[(f, n)] = dst
                    nc.vector.tensor_copy(out=dst[:], in_=ps[:])
                else:
                    nc.vector.tensor_copy(
                        out=kt_[f][:, n * 512:(n + 1) * 512], in_=ps[:])

            def v_group(n, t):
                ps = ps_m.tile([128, FPC], F32, tag="m1", name=f"v{t}")
                tloc = (t % 4) * 128
                for c in range(NC):
                    nc.tensor.matmul(
                        ps[:], xq[n][:, c, tloc:tloc + 128],
                        wv_sb[c][:], start=(c == 0), stop=(c == NC - 1))
                nc.vector.tensor_copy(
                    out=v65[t][:, :, 0:64],
                    in_=ps[:].rearrange("p (h d) -> p h d", h=HPC))

            def attention_hp(qc, hp):
                qsl = slice(qc * 512, (qc + 1) * 512)
                y_psA = ps_y.tile([65, 512], F32, tag="y")
                y_psB = ps_y.tile([65, 512], F32, tag="y")
                qtile = qtq[(hp, qc)]
                # off-diagonal k-tiles in order, then diagonal ones widest-
                # last so the accumulation group ends with a full-width
                # stop=True matmul (and starts with a bank-clearing one).
                kts = list(range(4 * qc)) + [4 * qc + d for d in (3, 2, 1, 0)]
                nk = 4 * qc + 4
                for i, kt in enumerate(kts):
                    first, last = (i == 0), (i == nk - 1)
                    d = kt - 4 * qc
                    dcol = 128 * d if d > 0 else 0
                    w = 512 - dcol
                    s_ps = ps_s.tile([128, 1024], F32, tag="s")
                    nc.tensor.matmul(
                        s_ps[:, dcol:512],
                        kt_[hp][0:64, kt * 128:(kt + 1) * 128],
                        qtile[0:64, dcol:512],
                        start=True, stop=True, tile_position=(0, 0))
                    nc.tensor.matmul(
                        s_ps[:, 512 + dcol:1024],
                        kt_[hp][64:128, kt * 128:(kt + 1) * 128],
                        qtile[64:128, dcol:512],
                        start=True, stop=True, tile_position=(64, 0))
                    pt = p_pt.tile([128, 1024], BF16, tag="pt")
                    pt3 = pt[:].rearrange("p (g q) -> p g q", g=2)
                    sp3 = s_ps[:].rearrange("p (g q) -> p g q", g=2)
                    nc.scalar.activation(
                        out=pt3[:, :, dcol:512], in_=sp3[:, :, dcol:512],
                        func=EXP, scale=float(SCALE))
                    if d >= 0:
                        pdiag = pt3[:, :, dcol:dcol + 128]
                        _, mbc = bass.broadcast_tensor_aps(
                            pdiag, trimask[:].rearrange("p (g q) -> p g q", g=1))
                        nc.vector.tensor_mul(pdiag, pdiag, mbc)
                    nc.tensor.matmul(
                        y_psA[:, dcol:512], v65[kt][:, 2 * hp, :],
                        pt[:, dcol:512],
                        start=first, stop=last)
                    nc.tensor.matmul(
                        y_psB[:, dcol:512], v65[kt][:, 2 * hp + 1, :],
                        pt[:, 512 + dcol:1024],
                        start=first, stop=last)

                # softmax division: reciprocal reads the denominator row
                # straight from PSUM; the numerator rows are staged to SBUF
                # (an engine op may have only ONE PSUM input)
                ytile = p_ytq.tile([128, 512], F32R, tag="yt",
                                   name=f"ytq{hp}_{qc}")
                ytq[(hp, qc)] = ytile

                recA = p_rec.tile([65, 512], F32R, tag="rec")
                with nc.allow_low_precision("f32r softmax denom reciprocal"):
                    nc.vector.reciprocal(out=recA[64:65, :], in_=y_psA[64:65, :])
                ysA = p_ys.tile([64, 512], F32, tag="ys")
                nc.vector.tensor_copy(out=ysA[:], in_=y_psA[0:64, :])
                bcA = ps_m.tile([64, 512], F32, tag="m1")
                nc.tensor.matmul(
                    bcA[:], ones_t[64:65, :], recA[64:65, :],
                    start=True, stop=True, tile_position=(64, 0))
                nc.vector.tensor_mul(ytile[0:64, :], ysA[:], bcA[:])

                recB = p_rec.tile([65, 512], F32R, tag="rec")
                with nc.allow_low_precision("f32r softmax denom reciprocal"):
                    nc.vector.reciprocal(out=recB[64:65, :], in_=y_psB[64:65, :])
                ysB = p_ys.tile([64, 512], F32, tag="ys")
                nc.vector.tensor_copy(out=ysB[:], in_=y_psB[0:64, :])
                bcB = ps_m.tile([64, 512], F32, tag="m1")
                nc.tensor.matmul(
                    bcB[:], ones_t[64:65, :], recB[64:65, :],
                    start=True, stop=True, tile_position=(64, 0))
                # head B writes directly into the upper partition half of the
                # assembled y^T tile (cross-partition-base DVE write) -- no
                # SBUF->SBUF assembly DMA on the attention->proj critical path
                nc.vector.tensor_mul(ytile[64:128, :], ysB[:], bcB[:])

            def proj_t(qc, t):
                tloc = (t - 4 * qc) * 128
                osb = p_osb.tile([128, C], F32, tag="osb", name=f"osb{t}")
                for nn in range(NN):
                    pj = ps_m.tile([128, npj], F32, tag="m1", name=f"pj{t}_{nn}")
                    for cf in range(NF):
                        nc.tensor.matmul(
                            pj[:],
                            ytq[(cf, qc)][:, tloc:tloc + 128],
                            wp_sb[cf][:, nn * npj:(nn + 1) * npj],
                            start=(cf == 0), stop=(cf == NF - 1))
                    nc.vector.tensor_copy(
                        out=osb[:, nn * npj:(nn + 1) * npj], in_=pj[:])
                nc.sync.dma_start(
                    out=out_d[t * 128:(t + 1) * 128, :], in_=osb[:])

            # software-pipelined emission: during attention of chunk qc
            # (ACT-exp-bound, PE under-occupied) we interleave the previous
            # chunk's projection AND the next chunk's QKV matmuls, so the
            # in-order PE queue always has data-ready filler work
            def qkv_chunk(n):
                for f in range(NF):
                    q_group(n, f, True)
                for f in range(NF):
                    q_group(n, f, False)
                for t in range(4 * n, 4 * n + 4):
                    v_group(n, t)

            if interleave_qkv:
                qkv_chunk(0)
            for qc in range(NQ):
                if qc + 1 < NQ:
                    dma_xq(qc + 1)
                if qc + 1 == NQ and prefetch_next:
                    dma_xq_tile(0, "xq0next")
                if not interleave_qkv:
                    qkv_chunk(qc)
                for hp in range(NF):
                    attention_hp(qc, hp)
                    if qc > 0:
                        proj_t(qc - 1, 4 * (qc - 1) + hp)
                    if interleave_qkv and qc + 1 < NQ:
                        q_group(qc + 1, hp, True)
                        q_group(qc + 1, hp, False)
                        v_group(qc + 1, 4 * (qc + 1) + hp)
            for t in range(4 * (NQ - 1), 4 * NQ):
                proj_t(NQ - 1, t)

        if loop:
            hints = tuple(
                (mybir.EngineType.PE, mybir.EngineType.DVE,
                 mybir.EngineType.Activation, mybir.EngineType.Pool,
                 mybir.EngineType.SP) if loop_hints else ())
            with tc.For_i(0, loop, 1, hint_engines=hints):
                for _ in range(body_reps):
                    emit_body(prefetch_next=True)
        else:
            emit_body()
    nc.finalize()
    return nc


def _make_masks():
    kk = np.arange(128)[:, None]
    jj = np.arange(128)[None, :]
    return (jj >= kk).astype(ml_dtypes.bfloat16)


def make_in_maps(x, W_qkv, W_proj):
    """Host-side sharding of full inputs into per-core input maps."""
    x = np.asarray(x, dtype=np.float32)
    W_qkv = np.asarray(W_qkv, dtype=np.float32)
    W_proj = np.asarray(W_proj, dtype=np.float32)
    masks = _make_masks()
    in_maps = []
    for core in range(NCORES):
        b, g = core // GROUPS, core % GROUPS
        in_maps.append({
            "xT": np.ascontiguousarray(x[b].T),
            "wq": np.ascontiguousarray(W_qkv[:, g * FPC:(g + 1) * FPC]),
            "wk": np.ascontiguousarray(W_qkv[:, C + g * FPC:C + (g + 1) * FPC]),
            "wv": np.ascontiguousarray(W_qkv[:, 2 * C + g * FPC:2 * C + (g + 1) * FPC]),
            "wp": np.ascontiguousarray(W_proj[g * FPC:(g + 1) * FPC, :]),
            "trimask": masks,
            "ones64": np.ones((1, 64), np.float32),
            "onesv": np.ones((128, HPC, 1), ml_dtypes.bfloat16),
        })
    return in_maps


_CACHE = {}


def _get_nc():
    if "nc" not in _CACHE:
        _CACHE["nc"] = build_nc()
    return _CACHE["nc"]


def run_cores(in_maps):
    res = run_bass_kernel_spmd(_get_nc(), in_maps, list(range(NCORES)))
    return res.results


def kernel(x, W_qkv, W_proj):
    results = run_cores(make_in_maps(x, W_qkv, W_proj))
    out = np.empty((B, T, C), dtype=np.float32)
    for b in range(B):
        out[b] = results[GROUPS * b]["out"]
        for g in range(1, GROUPS):
            out[b] += results[GROUPS * b + g]["out"]
    return out
